# revision 4
# baseline (speedup 1.0000x reference)
"""Varlen causal attention (flash_attn_varlen semantics) on 8 Trainium2 cores.

Sharding: 16 heads across 8 cores (2 heads/core, Ulysses-style head shard,
identity comms). Each core runs the same SPMD Bass program on its head slice.

Key design (transpose-free inner loop): compute S^T = K @ Q^T instead of
S = Q @ K^T.  Then P^T = exp(S^T * scale) comes out of the activation engine
already in [k, q] layout, which is exactly the stationary-operand layout the
PV matmul needs (lhsT = P^T chunk, rhs = V block) -- no P transposes at all.

Per head:
  prep: load Q,K,V (both heads fused per DMA, 1KB contiguous elements, Q/K
        issued before V); PE-transpose Q,K into [D, L] bf16; V + ones col.
  main: for each 256-row q superblock, for each in-mask k block j:
        S^T tile = K_j @ Q^T  (bf16, PSUM f32), exp on ScalarE (bf16 out),
        causal/segment masking on GpSimd, then PV matmuls accumulate
        O[q, 0:130] per 128-q chunk over j (col 0 = softmax denominator from
        a ones column in V).  Finalize: reciprocal + scale on DVE, DMA out.
The (I, j) tile list, trimmed to the causal x segment block mask, is
specialized on the host from cu_seqlens at trace time.  Width-2 tiles are
packed first so no S^T matmul straddles a PSUM bank.
"""

import numpy as np

L = 4096
H = 16
D = 128
N_CORES = 8
H_PER_CORE = H // N_CORES
SCALE = 1.0 / float(np.sqrt(D))
QB = 128          # q/k block size
SB = 2            # q blocks per superblock (256 q rows)
GROUP_UNITS = 8   # 128-col units per S^T PSUM group tile ([128,1024] f32)


def _seg_starts(cu: np.ndarray) -> np.ndarray:
    """Per-token segment start, exactly mirroring the reference searchsorted."""
    tok = np.arange(L)
    seg = np.searchsorted(cu[1:-1], tok, side="right")
    starts = np.concatenate([[0], cu[1:-1]])
    return starts[seg]


def _build_plan(cu: np.ndarray):
    """Host-side specialization of the block-sparse attention pattern.

    Returns a list (one entry per superblock I) of dicts:
      groups: list of groups; each group has
              runs:  [(u, j, i, n)]  one S^T matmul per run (n units wide)
              units: [(u, j, i)]     per-128-col bookkeeping
      masks:  list of ("tri"|"rows"|"zero", group_idx, unit_off, *args)
      pv:     {chunk i: [(group_idx, unit_off, j), ...]}
    """
    ss = _seg_starts(cu)
    n_qb = L // QB
    k_lo_b = [int(ss[i * QB]) // QB for i in range(n_qb)]
    bounds = [int(b) for b in cu[1:-1] if 0 < int(b) < L]

    plan = []
    for I in range(n_qb // SB):
        i0, i1 = SB * I, SB * I + SB - 1
        tiles = []
        for j in range(k_lo_b[i0], i1 + 1):
            qsb = max(i0, j)
            qeb = qsb
            for i in range(qsb, i1 + 1):
                if k_lo_b[i] <= j:
                    qeb = i + 1
                else:
                    break
            if qeb > qsb:
                tiles.append((j, qsb, qeb - qsb))
        # wide tiles first: keeps 2-unit tiles bank-aligned (no splits)
        tiles.sort(key=lambda t: (-t[2], t[0]))

        groups = [{"runs": [], "units": []}]
        masks = []
        pv = {i: [] for i in range(i0, i1 + 1)}
        cursor = 0
        for (j, qsb, n) in tiles:
            if cursor + n > GROUP_UNITS:
                groups.append({"runs": [], "units": []})
                cursor = 0
            g = len(groups) - 1
            u = cursor
            cursor += n
            groups[g]["runs"].append((u, j, qsb, n))
            for c in range(n):
                i = qsb + c
                uu = u + c
                groups[g]["units"].append((uu, j, i))
                pv[i].append((g, uu, j))
                if i == j:
                    masks.append(("tri", g, uu))
                q0u = i * QB
                for b in bounds:
                    if j * QB < b < (j + 1) * QB:
                        c0 = max(0, b - q0u)
                        rb = b - j * QB
                        if c0 < QB:
                            masks.append(("rows", g, uu, c0, rb))
                    elif (j + 1) * QB <= b:
                        c0 = b - q0u
                        if 0 <= c0 < QB:
                            masks.append(("zero", g, uu, c0))
        # PV accumulation order per chunk must be deterministic; sort by j so
        # start/stop flags are simply first/last of the list.
        for i in pv:
            pv[i].sort(key=lambda t: t[2])
        plan.append({"groups": groups, "masks": masks, "pv": pv,
                     "i0": i0, "n_chunks": i1 - i0 + 1})
    return plan


def _build(cu: np.ndarray):
    import concourse.mybir as mybir
    import concourse.tile as tile
    from concourse import bacc
    from concourse.masks import make_identity

    f32 = mybir.dt.float32
    bf16 = mybir.dt.bfloat16
    AF = mybir.ActivationFunctionType
    n_qb = L // QB
    plan = _build_plan(cu)

    nc = bacc.Bacc("TRN2", target_bir_lowering=False, debug=False,
                   num_devices=N_CORES)
    q_d = nc.dram_tensor("q", [L, H_PER_CORE, D], f32, kind="ExternalInput")
    k_d = nc.dram_tensor("k", [L, H_PER_CORE, D], f32, kind="ExternalInput")
    v_d = nc.dram_tensor("v", [L, H_PER_CORE, D], f32, kind="ExternalInput")
    o_d = nc.dram_tensor("out", [L, H_PER_CORE, D], f32, kind="ExternalOutput")

    with tile.TileContext(nc) as tc:
        with (
            tc.tile_pool(name="consts", bufs=1) as consts,
            tc.tile_pool(name="stage", bufs=1) as stage,
            tc.tile_pool(name="big", bufs=2) as big,
            tc.tile_pool(name="psb", bufs=10) as psb,
            tc.tile_pool(name="osb", bufs=2) as osb,
            tc.tile_pool(name="rsb", bufs=2) as rsb,
            tc.tile_pool(name="s_ps", bufs=2, space="PSUM") as s_ps,
            tc.tile_pool(name="o_ps", bufs=2, space="PSUM") as o_ps,
            tc.tile_pool(name="tr_ps", bufs=2, space="PSUM") as tr_ps,
        ):
            ident = consts.tile([128, 128], f32)
            make_identity(nc, ident[:])

            # ---- upfront DMA loads, both heads fused (contiguous 1KB per
            # (p, t) element), Q/K before V so transposes start early ----
            qs = stage.tile([128, n_qb, H_PER_CORE, D], f32, tag="qs")
            ks = stage.tile([128, n_qb, H_PER_CORE, D], f32, tag="ks")
            vs = stage.tile([128, n_qb, H_PER_CORE, D], f32, tag="vs")
            for b0 in range(0, n_qb, 8):
                r = slice(b0 * QB, (b0 + 8) * QB)
                nc.sync.dma_start(
                    qs[:, b0:b0 + 8, :, :],
                    q_d[r, :, :].rearrange("(t p) h d -> p t h d", p=128))
                nc.sync.dma_start(
                    ks[:, b0:b0 + 8, :, :],
                    k_d[r, :, :].rearrange("(t p) h d -> p t h d", p=128))
            for b0 in range(0, n_qb, 8):
                r = slice(b0 * QB, (b0 + 8) * QB)
                nc.sync.dma_start(
                    vs[:, b0:b0 + 8, :, :],
                    v_d[r, :, :].rearrange("(t p) h d -> p t h d", p=128))

            for h in range(H_PER_CORE):
                # ---- prep: V (+ones col) in bf16; Q,K transposed to [D, L]
                vA = big.tile([128, n_qb, 130], bf16, tag="vA")
                nc.gpsimd.memset(vA[:, :, 0:1], 1.0)
                for b0 in range(0, n_qb, 4):
                    nc.vector.tensor_copy(vA[:, b0:b0 + 4, 1:129],
                                          vs[:, b0:b0 + 4, h, :])
                qT = big.tile([128, L], bf16, tag="qT")
                kT = big.tile([128, L], bf16, tag="kT")
                for src, dstT in ((qs, qT), (ks, kT)):
                    for b0 in range(0, n_qb, 4):
                        trp = tr_ps.tile([128, 4, 128], f32, tag="tr")
                        for t in range(4):
                            nc.tensor.transpose(trp[:, t, :],
                                                src[:, b0 + t, h, :], ident[:])
                        nc.vector.tensor_copy(
                            dstT[:, b0 * QB:(b0 + 4) * QB], trp[:, :, :])

                # ---- main loop, software-pipelined by one superblock:
                # emit S^T+exp+masks for I, then PV+finalize for I-1.
                pending = None

                def emit_pv_finalize(pend):
                    I, ptiles = pend
                    sbp = plan[I]
                    i0 = sbp["i0"]
                    o_t = o_ps.tile([128, 512], f32, tag="o")
                    for c in range(sbp["n_chunks"]):
                        i = i0 + c
                        lst = sbp["pv"][i]
                        for nn, (g, u, j) in enumerate(lst):
                            nc.tensor.matmul(
                                o_t[:, c * 130:c * 130 + 130],
                                ptiles[g][:, u * QB:(u + 1) * QB],
                                vA[:, j, 0:130],
                                start=(nn == 0), stop=(nn == len(lst) - 1))
                    rec = rsb.tile([128, 2, 1], f32, tag="r")
                    den = o_t[:, 0:260].rearrange("p (c x) -> p c x", c=2)
                    nc.vector.reciprocal(rec[:, :, :], den[:, :, 0:1])
                    ost = osb.tile([128, 2, 128], f32, tag="ost")
                    for c in range(sbp["n_chunks"]):
                        nc.vector.tensor_scalar_mul(
                            ost[:, c, :], o_t[:, c * 130 + 1:c * 130 + 129],
                            rec[:, c, :])
                    nc.gpsimd.dma_start(
                        o_d[i0 * QB:(i0 + SB) * QB, h, :].rearrange(
                            "(t p) d -> p t d", p=128),
                        ost[:, :, :])

                for I, sbp in enumerate(plan):
                    ptiles = []
                    for group in sbp["groups"]:
                        if not group["units"]:
                            continue
                        s_t = s_ps.tile([128, 1024], f32, tag="s")
                        p_t = psb.tile([128, 1024], bf16, tag="p")
                        for (u, j, i, n) in group["runs"]:
                            nc.tensor.matmul(
                                s_t[:, u * QB:(u + n) * QB],
                                kT[:, j * QB:(j + 1) * QB],
                                qT[:, i * QB:(i + n) * QB],
                                start=True, stop=True)
                        gw = (group["units"][-1][0] + 1) * QB
                        nc.scalar.activation(p_t[:, 0:gw], s_t[:, 0:gw],
                                             AF.Exp, scale=SCALE)
                        ptiles.append(p_t)
                    # masks (gpsimd), after exp
                    for m in sbp["masks"]:
                        kind, g, u = m[0], m[1], m[2]
                        p_t = ptiles[g]
                        sl = p_t[:, u * QB:(u + 1) * QB]
                        if kind == "tri":
                            # keep q >= k: iota = -p + c >= 0
                            nc.gpsimd.affine_select(
                                out=sl, in_=sl,
                                compare_op=mybir.AluOpType.is_ge, fill=0.0,
                                base=0, pattern=[[1, QB]],
                                channel_multiplier=-1)
                        elif kind == "rows":
                            c0, rb = m[3], m[4]
                            sl2 = p_t[:, u * QB + c0:(u + 1) * QB]
                            # keep k-rows >= rb: iota = p - rb >= 0
                            nc.gpsimd.affine_select(
                                out=sl2, in_=sl2,
                                compare_op=mybir.AluOpType.is_ge, fill=0.0,
                                base=-rb, pattern=[[0, QB - c0]],
                                channel_multiplier=1)
                        else:  # "zero"
                            c0 = m[3]
                            nc.gpsimd.memset(p_t[:, u * QB + c0:(u + 1) * QB],
                                             0.0)
                    if pending is not None:
                        emit_pv_finalize(pending)
                    pending = (I, ptiles)
                emit_pv_finalize(pending)
                pending = None

    nc.compile()
    return nc


def _run(query, key, value, cu_seqlens, trace=False, **spmd_kwargs):
    from concourse import bass_utils

    query = np.ascontiguousarray(np.asarray(query, dtype=np.float32))
    key = np.ascontiguousarray(np.asarray(key, dtype=np.float32))
    value = np.ascontiguousarray(np.asarray(value, dtype=np.float32))
    cu = np.asarray(cu_seqlens, dtype=np.int64)

    nc = _build(cu)
    in_maps = []
    for c in range(N_CORES):
        hs = slice(c * H_PER_CORE, (c + 1) * H_PER_CORE)
        in_maps.append({
            "q": np.ascontiguousarray(query[:, hs, :]),
            "k": np.ascontiguousarray(key[:, hs, :]),
            "v": np.ascontiguousarray(value[:, hs, :]),
        })
    res = bass_utils.run_bass_kernel_spmd(nc, in_maps, list(range(N_CORES)),
                                          trace=trace, **spmd_kwargs)
    out = np.empty((L, H, D), dtype=np.float32)
    for c in range(N_CORES):
        out[:, c * H_PER_CORE:(c + 1) * H_PER_CORE, :] = res.results[c]["out"]
    return out, res


def kernel(query, key, value, cu_seqlens):
    out, _ = _run(query, key, value, cu_seqlens)
    return out


# revision 9
# speedup vs baseline: 1.1294x; 1.1294x over previous
"""Varlen causal attention (flash_attn_varlen semantics) on 8 Trainium2 cores.

Sharding: 16 heads across 8 cores (2 heads/core, Ulysses-style head shard,
identity comms). Each core runs the same SPMD Bass program on its head slice.

Key design (transpose-free inner loop): compute S^T = K @ Q^T instead of
S = Q @ K^T.  Then P^T = exp(S^T * scale) comes out of the activation engine
already in [k, q] layout, which is exactly the stationary-operand layout the
PV matmul needs (lhsT = P^T chunk, rhs = V block) -- no P transposes at all.

Per head:
  prep: load Q,K,V (both heads fused per DMA, 1KB contiguous elements, Q/K
        issued before V); PE-transpose Q,K into [D, L] bf16; V + ones col.
  main: for each 256-row q superblock, for each in-mask k block j:
        S^T tile = K_j @ Q^T  (bf16, PSUM f32), exp on ScalarE (bf16 out),
        causal/segment masking on GpSimd, then PV matmuls accumulate
        O[q, 0:130] per 128-q chunk over j (col 0 = softmax denominator from
        a ones column in V).  Finalize: reciprocal + scale on DVE, DMA out.
The (I, j) tile list, trimmed to the causal x segment block mask, is
specialized on the host from cu_seqlens at trace time.  Width-2 tiles are
packed first so no S^T matmul straddles a PSUM bank.
"""

import numpy as np

L = 4096
H = 16
D = 128
N_CORES = 8
H_PER_CORE = H // N_CORES
SCALE = 1.0 / float(np.sqrt(D))
QB = 128          # q/k block size
SB = 2            # q blocks per superblock (256 q rows)
GROUP_UNITS = 8   # 128-col units per S^T PSUM group tile ([128,1024] f32)


def _seg_starts(cu: np.ndarray) -> np.ndarray:
    """Per-token segment start, exactly mirroring the reference searchsorted."""
    tok = np.arange(L)
    seg = np.searchsorted(cu[1:-1], tok, side="right")
    starts = np.concatenate([[0], cu[1:-1]])
    return starts[seg]


def _build_plan(cu: np.ndarray):
    """Host-side specialization of the block-sparse attention pattern.

    Returns a list (one entry per superblock I) of dicts:
      groups: list of groups; each group has
              runs:  [(u, j, i, n)]  one S^T matmul per run (n units wide)
              units: [(u, j, i)]     per-128-col bookkeeping
      masks:  list of ("tri"|"rows"|"zero", group_idx, unit_off, *args)
      pv:     {chunk i: [(group_idx, unit_off, j), ...]}
    """
    ss = _seg_starts(cu)
    n_qb = L // QB
    k_lo_b = [int(ss[i * QB]) // QB for i in range(n_qb)]
    bounds = [int(b) for b in cu[1:-1] if 0 < int(b) < L]

    plan = []
    for I in range(n_qb // SB):
        i0, i1 = SB * I, SB * I + SB - 1
        tiles = []
        for j in range(k_lo_b[i0], i1 + 1):
            qsb = max(i0, j)
            qeb = qsb
            for i in range(qsb, i1 + 1):
                if k_lo_b[i] <= j:
                    qeb = i + 1
                else:
                    break
            if qeb > qsb:
                tiles.append((j, qsb, qeb - qsb))
        # wide tiles first: keeps 2-unit tiles bank-aligned (no splits)
        tiles.sort(key=lambda t: (-t[2], t[0]))

        groups = [{"runs": [], "units": []}]
        masks = []
        pv = {i: [] for i in range(i0, i1 + 1)}
        cursor = 0
        for (j, qsb, n) in tiles:
            if cursor + n > GROUP_UNITS:
                groups.append({"runs": [], "units": []})
                cursor = 0
            g = len(groups) - 1
            u = cursor
            cursor += n
            groups[g]["runs"].append((u, j, qsb, n))
            for c in range(n):
                i = qsb + c
                uu = u + c
                groups[g]["units"].append((uu, j, i))
                pv[i].append((g, uu, j))
                if i == j:
                    masks.append(("tri", g, uu))
                q0u = i * QB
                for b in bounds:
                    if j * QB < b < (j + 1) * QB:
                        c0 = max(0, b - q0u)
                        rb = b - j * QB
                        if c0 < QB:
                            masks.append(("rows", g, uu, c0, rb))
                    elif (j + 1) * QB <= b:
                        c0 = b - q0u
                        if 0 <= c0 < QB:
                            masks.append(("zero", g, uu, c0))
        # PV accumulation order per chunk must be deterministic; sort by j so
        # start/stop flags are simply first/last of the list.
        for i in pv:
            pv[i].sort(key=lambda t: t[2])
        plan.append({"groups": groups, "masks": masks, "pv": pv,
                     "i0": i0, "n_chunks": i1 - i0 + 1})
    return plan


def _build(cu: np.ndarray):
    import concourse.mybir as mybir
    import concourse.tile as tile
    from concourse import bacc
    from concourse.masks import make_identity

    f32 = mybir.dt.float32
    bf16 = mybir.dt.bfloat16
    AF = mybir.ActivationFunctionType
    n_qb = L // QB
    plan = _build_plan(cu)

    nc = bacc.Bacc("TRN2", target_bir_lowering=False, debug=False,
                   num_devices=N_CORES)
    q_d = nc.dram_tensor("q", [L, H_PER_CORE, D], f32, kind="ExternalInput")
    k_d = nc.dram_tensor("k", [L, H_PER_CORE, D], f32, kind="ExternalInput")
    v_d = nc.dram_tensor("v", [L, H_PER_CORE, D], f32, kind="ExternalInput")
    o_d = nc.dram_tensor("out", [L, H_PER_CORE, D], f32, kind="ExternalOutput")

    with tile.TileContext(nc) as tc:
        with (
            tc.tile_pool(name="consts", bufs=1) as consts,
            tc.tile_pool(name="stage", bufs=1) as stage,
            tc.tile_pool(name="big", bufs=2) as big,
            tc.tile_pool(name="psb", bufs=10) as psb,
            tc.tile_pool(name="osb", bufs=2) as osb,
            tc.tile_pool(name="rsb", bufs=2) as rsb,
            tc.tile_pool(name="s_ps", bufs=2, space="PSUM") as s_ps,
            tc.tile_pool(name="o_ps", bufs=2, space="PSUM") as o_ps,
            tc.tile_pool(name="tr_ps", bufs=2, space="PSUM") as tr_ps,
        ):
            ident = consts.tile([128, 128], f32)
            make_identity(nc, ident[:])

            # ---- DMA loads: per head, Q/K/V 8-block spans interleaved so
            # the earliest blocks of every tensor land first ----
            stages = {}
            for h in range(H_PER_CORE):
                qs = stage.tile([128, n_qb, D], f32, tag=f"qs{h}")
                ks = stage.tile([128, n_qb, D], f32, tag=f"ks{h}")
                vs = stage.tile([128, n_qb, D], f32, tag=f"vs{h}")
                for b0 in range(0, n_qb, 8):
                    r = slice(b0 * QB, (b0 + 8) * QB)
                    for t_d, t_s in ((qs, q_d), (ks, k_d), (vs, v_d)):
                        nc.sync.dma_start(
                            t_d[:, b0:b0 + 8, :],
                            t_s[r, h, :].rearrange("(t p) d -> p t d", p=128))
                stages[h] = (qs, ks, vs)

            for h in range(H_PER_CORE):
                qs, ks, vs = stages[h]
                # prep targets; transposes + V casts are emitted on demand
                # inside the main loop so compute starts with the first spans
                vA = big.tile([128, n_qb, 130], bf16, tag="vA")
                nc.gpsimd.memset(vA[:, :, 0:1], 1.0)
                qT = big.tile([128, L], bf16, tag="qT")
                kT = big.tile([128, L], bf16, tag="kT")
                prep_done = [0]   # blocks transposed / V-cast so far

                def emit_prep(need_b):
                    while prep_done[0] < min(need_b, n_qb):
                        b0 = prep_done[0]
                        for src, dstT in ((qs, qT), (ks, kT)):
                            trp = tr_ps.tile([128, 4, 128], f32, tag="tr")
                            for t in range(4):
                                nc.tensor.transpose(trp[:, t, :],
                                                    src[:, b0 + t, :],
                                                    ident[:])
                            nc.vector.tensor_copy(
                                dstT[:, b0 * QB:(b0 + 4) * QB], trp[:, :, :])
                        nc.vector.tensor_copy(vA[:, b0:b0 + 4, 1:129],
                                              vs[:, b0:b0 + 4, :])
                        prep_done[0] += 4

                # ---- main loop, software-pipelined by one superblock:
                # emit S^T+exp+masks for I, then PV+finalize for I-1.
                pending = None

                ost_state = {"tile": None, "i0": 0, "filled": 0}

                def flush_out():
                    nf = ost_state["filled"]
                    if not nf:
                        return
                    i0 = ost_state["i0"]
                    nc.sync.dma_start(
                        o_d[i0 * QB:(i0 + nf) * QB, h, :].rearrange(
                            "(t p) d -> p t d", p=128),
                        ost_state["tile"][:, 0:nf, :])
                    ost_state["tile"] = None
                    ost_state["filled"] = 0

                def emit_pv_finalize(pend):
                    I, ptiles = pend
                    sbp = plan[I]
                    i0 = sbp["i0"]
                    o_t = o_ps.tile([128, 512], f32, tag="o")
                    for c in range(sbp["n_chunks"]):
                        i = i0 + c
                        lst = sbp["pv"][i]
                        for nn, (g, u, j) in enumerate(lst):
                            nc.tensor.matmul(
                                o_t[:, c * 130:c * 130 + 130],
                                ptiles[g][:, u * QB:(u + 1) * QB],
                                vA[:, j, 0:130],
                                start=(nn == 0), stop=(nn == len(lst) - 1))
                    rec = rsb.tile([128, 2, 1], f32, tag="r")
                    den = o_t[:, 0:260].rearrange("p (c x) -> p c x", c=2)
                    nc.vector.reciprocal(rec[:, :, :], den[:, :, 0:1])
                    if ost_state["tile"] is None:
                        ost_state["tile"] = osb.tile([128, 2 * SB, 128], f32,
                                                     tag="ost", name="ost")
                        ost_state["i0"] = i0
                    ost = ost_state["tile"]
                    for c in range(sbp["n_chunks"]):
                        nc.vector.tensor_scalar_mul(
                            ost[:, ost_state["filled"] + c, :],
                            o_t[:, c * 130 + 1:c * 130 + 129],
                            rec[:, c, :])
                    ost_state["filled"] += sbp["n_chunks"]
                    if ost_state["filled"] >= 2 * SB:
                        flush_out()

                for I, sbp in enumerate(plan):
                    emit_prep(sbp["i0"] + sbp["n_chunks"])
                    ptiles = []
                    for group in sbp["groups"]:
                        if not group["units"]:
                            continue
                        s_t = s_ps.tile([128, 1024], f32, tag="s")
                        p_t = psb.tile([128, 1024], bf16, tag="p")
                        for (u, j, i, n) in group["runs"]:
                            nc.tensor.matmul(
                                s_t[:, u * QB:(u + n) * QB],
                                kT[:, j * QB:(j + 1) * QB],
                                qT[:, i * QB:(i + n) * QB],
                                start=True, stop=True)
                        gw = (group["units"][-1][0] + 1) * QB
                        nc.scalar.activation(p_t[:, 0:gw], s_t[:, 0:gw],
                                             AF.Exp, scale=SCALE)
                        ptiles.append(p_t)
                    # masks (gpsimd), after exp
                    for m in sbp["masks"]:
                        kind, g, u = m[0], m[1], m[2]
                        p_t = ptiles[g]
                        sl = p_t[:, u * QB:(u + 1) * QB]
                        if kind == "tri":
                            # keep q >= k: iota = -p + c >= 0
                            nc.gpsimd.affine_select(
                                out=sl, in_=sl,
                                compare_op=mybir.AluOpType.is_ge, fill=0.0,
                                base=0, pattern=[[1, QB]],
                                channel_multiplier=-1)
                        elif kind == "rows":
                            c0, rb = m[3], m[4]
                            sl2 = p_t[:, u * QB + c0:(u + 1) * QB]
                            # keep k-rows >= rb: iota = p - rb >= 0
                            nc.gpsimd.affine_select(
                                out=sl2, in_=sl2,
                                compare_op=mybir.AluOpType.is_ge, fill=0.0,
                                base=-rb, pattern=[[0, QB - c0]],
                                channel_multiplier=1)
                        else:  # "zero"
                            c0 = m[3]
                            nc.gpsimd.memset(p_t[:, u * QB + c0:(u + 1) * QB],
                                             0.0)
                    if pending is not None:
                        emit_pv_finalize(pending)
                    pending = (I, ptiles)
                emit_pv_finalize(pending)
                flush_out()
                pending = None

    nc.compile()
    return nc


def _run(query, key, value, cu_seqlens, trace=False, **spmd_kwargs):
    from concourse import bass_utils

    query = np.ascontiguousarray(np.asarray(query, dtype=np.float32))
    key = np.ascontiguousarray(np.asarray(key, dtype=np.float32))
    value = np.ascontiguousarray(np.asarray(value, dtype=np.float32))
    cu = np.asarray(cu_seqlens, dtype=np.int64)

    nc = _build(cu)
    in_maps = []
    for c in range(N_CORES):
        hs = slice(c * H_PER_CORE, (c + 1) * H_PER_CORE)
        in_maps.append({
            "q": np.ascontiguousarray(query[:, hs, :]),
            "k": np.ascontiguousarray(key[:, hs, :]),
            "v": np.ascontiguousarray(value[:, hs, :]),
        })
    res = bass_utils.run_bass_kernel_spmd(nc, in_maps, list(range(N_CORES)),
                                          trace=trace, **spmd_kwargs)
    out = np.empty((L, H, D), dtype=np.float32)
    for c in range(N_CORES):
        out[:, c * H_PER_CORE:(c + 1) * H_PER_CORE, :] = res.results[c]["out"]
    return out, res


def kernel(query, key, value, cu_seqlens):
    out, _ = _run(query, key, value, cu_seqlens)
    return out


# revision 13
# speedup vs baseline: 1.1357x; 1.0056x over previous
"""Varlen causal attention (flash_attn_varlen semantics) on 8 Trainium2 cores.

Sharding: 16 heads across 8 cores (2 heads/core, Ulysses-style head shard,
identity comms). Each core runs the same SPMD Bass program on its head slice.

Key design (transpose-free inner loop): compute S^T = K @ Q^T instead of
S = Q @ K^T.  Then P^T = exp(S^T * scale) comes out of the activation engine
already in [k, q] layout, which is exactly the stationary-operand layout the
PV matmul needs (lhsT = P^T chunk, rhs = V block) -- no P transposes at all.

Per head:
  prep: load Q,K,V (both heads fused per DMA, 1KB contiguous elements, Q/K
        issued before V); PE-transpose Q,K into [D, L] bf16; V + ones col.
  main: for each 256-row q superblock, for each in-mask k block j:
        S^T tile = K_j @ Q^T  (bf16, PSUM f32), exp on ScalarE (bf16 out),
        causal/segment masking on GpSimd, then PV matmuls accumulate
        O[q, 0:130] per 128-q chunk over j (col 0 = softmax denominator from
        a ones column in V).  Finalize: reciprocal + scale on DVE, DMA out.
The (I, j) tile list, trimmed to the causal x segment block mask, is
specialized on the host from cu_seqlens at trace time.  Width-2 tiles are
packed first so no S^T matmul straddles a PSUM bank.
"""

import numpy as np

L = 4096
H = 16
D = 128
N_CORES = 8
H_PER_CORE = H // N_CORES
SCALE = 1.0 / float(np.sqrt(D))
QB = 128          # q/k block size
SB = 2            # q blocks per superblock (256 q rows)
GROUP_UNITS = 8   # 128-col units per S^T PSUM group tile ([128,1024] f32)


def _seg_starts(cu: np.ndarray) -> np.ndarray:
    """Per-token segment start, exactly mirroring the reference searchsorted."""
    tok = np.arange(L)
    seg = np.searchsorted(cu[1:-1], tok, side="right")
    starts = np.concatenate([[0], cu[1:-1]])
    return starts[seg]


def _build_plan(cu: np.ndarray):
    """Host-side specialization of the block-sparse attention pattern.

    Returns a list (one entry per superblock I) of dicts:
      groups: list of groups; each group has
              runs:  [(u, j, i, n)]  one S^T matmul per run (n units wide)
              units: [(u, j, i)]     per-128-col bookkeeping
      masks:  list of ("tri"|"rows"|"zero", group_idx, unit_off, *args)
      pv:     {chunk i: [(group_idx, unit_off, j), ...]}
    """
    ss = _seg_starts(cu)
    n_qb = L // QB
    k_lo_b = [int(ss[i * QB]) // QB for i in range(n_qb)]
    bounds = [int(b) for b in cu[1:-1] if 0 < int(b) < L]

    plan = []
    for I in range(n_qb // SB):
        i0, i1 = SB * I, SB * I + SB - 1
        tiles = []
        for j in range(k_lo_b[i0], i1 + 1):
            qsb = max(i0, j)
            qeb = qsb
            for i in range(qsb, i1 + 1):
                if k_lo_b[i] <= j:
                    qeb = i + 1
                else:
                    break
            if qeb > qsb:
                tiles.append((j, qsb, qeb - qsb))
        # wide tiles first: keeps 2-unit tiles bank-aligned (no splits)
        tiles.sort(key=lambda t: (-t[2], t[0]))

        groups = [{"runs": [], "units": []}]
        masks = []
        pv = {i: [] for i in range(i0, i1 + 1)}
        cursor = 0
        for (j, qsb, n) in tiles:
            if cursor + n > GROUP_UNITS:
                groups.append({"runs": [], "units": []})
                cursor = 0
            g = len(groups) - 1
            u = cursor
            cursor += n
            groups[g]["runs"].append((u, j, qsb, n))
            for c in range(n):
                i = qsb + c
                uu = u + c
                groups[g]["units"].append((uu, j, i))
                pv[i].append((g, uu, j))
                if i == j:
                    masks.append(("tri", g, uu))
                q0u = i * QB
                for b in bounds:
                    if j * QB < b < (j + 1) * QB:
                        c0 = max(0, b - q0u)
                        rb = b - j * QB
                        if c0 < QB:
                            masks.append(("rows", g, uu, c0, rb))
                    elif (j + 1) * QB <= b:
                        c0 = b - q0u
                        if 0 <= c0 < QB:
                            masks.append(("zero", g, uu, c0))
        # PV accumulation order per chunk must be deterministic; sort by j so
        # start/stop flags are simply first/last of the list.
        for i in pv:
            pv[i].sort(key=lambda t: t[2])
        plan.append({"groups": groups, "masks": masks, "pv": pv,
                     "i0": i0, "n_chunks": i1 - i0 + 1})
    return plan


def _build(cu: np.ndarray):
    import concourse.mybir as mybir
    import concourse.tile as tile
    from concourse import bacc
    from concourse.masks import make_identity

    f32 = mybir.dt.float32
    bf16 = mybir.dt.bfloat16
    AF = mybir.ActivationFunctionType
    n_qb = L // QB
    plan = _build_plan(cu)

    nc = bacc.Bacc("TRN2", target_bir_lowering=False, debug=False,
                   num_devices=N_CORES)
    q_d = nc.dram_tensor("q", [L, H_PER_CORE, D], f32, kind="ExternalInput")
    k_d = nc.dram_tensor("k", [L, H_PER_CORE, D], f32, kind="ExternalInput")
    v_d = nc.dram_tensor("v", [L, H_PER_CORE, D], f32, kind="ExternalInput")
    o_d = nc.dram_tensor("out", [L, H_PER_CORE, D], f32, kind="ExternalOutput")

    with tile.TileContext(nc) as tc:
        with (
            tc.tile_pool(name="consts", bufs=1) as consts,
            tc.tile_pool(name="stage", bufs=1) as stage,
            tc.tile_pool(name="big", bufs=1) as big,
            tc.tile_pool(name="psb", bufs=10) as psb,
            tc.tile_pool(name="osb", bufs=2) as osb,
            tc.tile_pool(name="rsb", bufs=2) as rsb,
            tc.tile_pool(name="s_ps", bufs=2, space="PSUM") as s_ps,
            tc.tile_pool(name="o_ps", bufs=2, space="PSUM") as o_ps,
            tc.tile_pool(name="tr_ps", bufs=2, space="PSUM") as tr_ps,
        ):
            ident = consts.tile([128, 128], f32)
            make_identity(nc, ident[:])

            # ---- DMA loads: per head, Q/K/V 8-block spans interleaved so
            # the earliest blocks of every tensor land first ----
            stages = {}
            for h in range(H_PER_CORE):
                qs = stage.tile([128, n_qb, D], f32, tag=f"qs{h}")
                ks = stage.tile([128, n_qb, D], f32, tag=f"ks{h}")
                vs = stage.tile([128, n_qb, D], f32, tag=f"vs{h}")
                for b0 in range(0, n_qb, 8):
                    r = slice(b0 * QB, (b0 + 8) * QB)
                    for t_d, t_s in ((qs, q_d), (ks, k_d), (vs, v_d)):
                        nc.sync.dma_start(
                            t_d[:, b0:b0 + 8, :],
                            t_s[r, h, :].rearrange("(t p) d -> p t d", p=128))
                stages[h] = (qs, ks, vs)

            # per-head prep state; transposes + V casts are emitted on demand
            # inside the main loop (and dripped ahead for the next head) so
            # the PE never waits on a bulk prep phase
            hstate = []
            for h in range(H_PER_CORE):
                vA = big.tile([128, n_qb, 130], bf16, tag=f"vA{h}")
                nc.gpsimd.memset(vA[:, :, 0:1], 1.0)
                qT = big.tile([128, L], bf16, tag=f"qT{h}")
                kT = big.tile([128, L], bf16, tag=f"kT{h}")
                hstate.append({"vA": vA, "qT": qT, "kT": kT, "done": 0})

            def emit_prep_batch(h):
                hs = hstate[h]
                b0 = hs["done"]
                qs, ks, vs = stages[h]
                for src, dstT in ((qs, hs["qT"]), (ks, hs["kT"])):
                    trp = tr_ps.tile([128, 4, 128], f32, tag="tr")
                    for t in range(4):
                        nc.tensor.transpose(trp[:, t, :], src[:, b0 + t, :],
                                            ident[:])
                    nc.vector.tensor_copy(
                        dstT[:, b0 * QB:(b0 + 4) * QB], trp[:, :, :])
                nc.vector.tensor_copy(hs["vA"][:, b0:b0 + 4, 1:129],
                                      vs[:, b0:b0 + 4, :])
                hs["done"] += 4

            def emit_prep(h, need_b):
                while hstate[h]["done"] < min(need_b, n_qb):
                    emit_prep_batch(h)

            for h in range(H_PER_CORE):
                vA = hstate[h]["vA"]
                qT = hstate[h]["qT"]
                kT = hstate[h]["kT"]

                # ---- main loop, software-pipelined by one superblock:
                # emit S^T+exp+masks for I, then PV+finalize for I-1.
                pending = None

                ost_state = {"tile": None, "i0": 0, "filled": 0}

                def flush_out():
                    nf = ost_state["filled"]
                    if not nf:
                        return
                    i0 = ost_state["i0"]
                    nc.sync.dma_start(
                        o_d[i0 * QB:(i0 + nf) * QB, h, :].rearrange(
                            "(t p) d -> p t d", p=128),
                        ost_state["tile"][:, 0:nf, :])
                    ost_state["tile"] = None
                    ost_state["filled"] = 0

                def emit_pv_finalize(pend):
                    I, ptiles = pend
                    sbp = plan[I]
                    i0 = sbp["i0"]
                    o_t = o_ps.tile([128, 512], f32, tag="o")
                    for c in range(sbp["n_chunks"]):
                        i = i0 + c
                        lst = sbp["pv"][i]
                        for nn, (g, u, j) in enumerate(lst):
                            nc.tensor.matmul(
                                o_t[:, c * 130:c * 130 + 130],
                                ptiles[g][:, u * QB:(u + 1) * QB],
                                vA[:, j, 0:130],
                                start=(nn == 0), stop=(nn == len(lst) - 1))
                    rec = rsb.tile([128, 2, 1], f32, tag="r")
                    den = o_t[:, 0:260].rearrange("p (c x) -> p c x", c=2)
                    nc.vector.reciprocal(rec[:, :, :], den[:, :, 0:1])
                    if ost_state["tile"] is None:
                        ost_state["tile"] = osb.tile([128, 2 * SB, 128], f32,
                                                     tag="ost", name="ost")
                        ost_state["i0"] = i0
                    ost = ost_state["tile"]
                    for c in range(sbp["n_chunks"]):
                        nc.vector.tensor_scalar_mul(
                            ost[:, ost_state["filled"] + c, :],
                            o_t[:, c * 130 + 1:c * 130 + 129],
                            rec[:, c, :])
                    ost_state["filled"] += sbp["n_chunks"]
                    if ost_state["filled"] >= 2 * SB:
                        flush_out()

                n_sb = len(plan)
                for I, sbp in enumerate(plan):
                    emit_prep(h, sbp["i0"] + sbp["n_chunks"])
                    # drip next head's prep into this head's second half
                    if h + 1 < H_PER_CORE and I >= n_sb // 2:
                        nxt = hstate[h + 1]
                        if nxt["done"] < n_qb:
                            emit_prep_batch(h + 1)
                    ptiles = []
                    for group in sbp["groups"]:
                        if not group["units"]:
                            continue
                        s_t = s_ps.tile([128, 1024], f32, tag="s")
                        p_t = psb.tile([128, 1024], bf16, tag="p")
                        for (u, j, i, n) in group["runs"]:
                            nc.tensor.matmul(
                                s_t[:, u * QB:(u + n) * QB],
                                kT[:, j * QB:(j + 1) * QB],
                                qT[:, i * QB:(i + n) * QB],
                                start=True, stop=True)
                        gw = (group["units"][-1][0] + 1) * QB
                        nc.scalar.activation(p_t[:, 0:gw], s_t[:, 0:gw],
                                             AF.Exp, scale=SCALE)
                        ptiles.append(p_t)
                    # masks (gpsimd), after exp
                    for m in sbp["masks"]:
                        kind, g, u = m[0], m[1], m[2]
                        p_t = ptiles[g]
                        sl = p_t[:, u * QB:(u + 1) * QB]
                        if kind == "tri":
                            # keep q >= k: iota = -p + c >= 0
                            nc.gpsimd.affine_select(
                                out=sl, in_=sl,
                                compare_op=mybir.AluOpType.is_ge, fill=0.0,
                                base=0, pattern=[[1, QB]],
                                channel_multiplier=-1)
                        elif kind == "rows":
                            c0, rb = m[3], m[4]
                            sl2 = p_t[:, u * QB + c0:(u + 1) * QB]
                            # keep k-rows >= rb: iota = p - rb >= 0
                            nc.gpsimd.affine_select(
                                out=sl2, in_=sl2,
                                compare_op=mybir.AluOpType.is_ge, fill=0.0,
                                base=-rb, pattern=[[0, QB - c0]],
                                channel_multiplier=1)
                        else:  # "zero"
                            c0 = m[3]
                            nc.gpsimd.memset(p_t[:, u * QB + c0:(u + 1) * QB],
                                             0.0)
                    if pending is not None:
                        emit_pv_finalize(pending)
                    pending = (I, ptiles)
                emit_pv_finalize(pending)
                flush_out()
                pending = None

    nc.compile()
    return nc


def _run(query, key, value, cu_seqlens, trace=False, **spmd_kwargs):
    from concourse import bass_utils

    query = np.ascontiguousarray(np.asarray(query, dtype=np.float32))
    key = np.ascontiguousarray(np.asarray(key, dtype=np.float32))
    value = np.ascontiguousarray(np.asarray(value, dtype=np.float32))
    cu = np.asarray(cu_seqlens, dtype=np.int64)

    nc = _build(cu)
    in_maps = []
    for c in range(N_CORES):
        hs = slice(c * H_PER_CORE, (c + 1) * H_PER_CORE)
        in_maps.append({
            "q": np.ascontiguousarray(query[:, hs, :]),
            "k": np.ascontiguousarray(key[:, hs, :]),
            "v": np.ascontiguousarray(value[:, hs, :]),
        })
    res = bass_utils.run_bass_kernel_spmd(nc, in_maps, list(range(N_CORES)),
                                          trace=trace, **spmd_kwargs)
    out = np.empty((L, H, D), dtype=np.float32)
    for c in range(N_CORES):
        out[:, c * H_PER_CORE:(c + 1) * H_PER_CORE, :] = res.results[c]["out"]
    return out, res


def kernel(query, key, value, cu_seqlens):
    out, _ = _run(query, key, value, cu_seqlens)
    return out


# revision 15
# speedup vs baseline: 1.3781x; 1.2134x over previous
"""Varlen causal attention (flash_attn_varlen semantics) on 8 Trainium2 cores.

Sharding: 16 heads across 8 cores (2 heads/core, Ulysses-style head shard,
identity comms). Each core runs the same SPMD Bass program on its head slice.

Key design (transpose-free inner loop): compute S^T = K @ Q^T instead of
S = Q @ K^T.  Then P^T = exp(S^T * scale) comes out of the activation engine
already in [k, q] layout, which is exactly the stationary-operand layout the
PV matmul needs (lhsT = P^T chunk, rhs = V block) -- no P transposes at all.

Per head:
  prep: load Q,K,V (both heads fused per DMA, 1KB contiguous elements, Q/K
        issued before V); PE-transpose Q,K into [D, L] bf16; V + ones col.
  main: for each 256-row q superblock, for each in-mask k block j:
        S^T tile = K_j @ Q^T  (bf16, PSUM f32), exp on ScalarE (bf16 out),
        causal/segment masking on GpSimd, then PV matmuls accumulate
        O[q, 0:130] per 128-q chunk over j (col 0 = softmax denominator from
        a ones column in V).  Finalize: reciprocal + scale on DVE, DMA out.
The (I, j) tile list, trimmed to the causal x segment block mask, is
specialized on the host from cu_seqlens at trace time.  Width-2 tiles are
packed first so no S^T matmul straddles a PSUM bank.
"""

import numpy as np

L = 4096
H = 16
D = 128
N_CORES = 8
H_PER_CORE = H // N_CORES
SCALE = 1.0 / float(np.sqrt(D))
QB = 128          # q/k block size
SB = 2            # q blocks per superblock (256 q rows)
GROUP_UNITS = 8   # 128-col units per S^T PSUM group tile ([128,1024] f32)


def _seg_starts(cu: np.ndarray) -> np.ndarray:
    """Per-token segment start, exactly mirroring the reference searchsorted."""
    tok = np.arange(L)
    seg = np.searchsorted(cu[1:-1], tok, side="right")
    starts = np.concatenate([[0], cu[1:-1]])
    return starts[seg]


def _build_plan(cu: np.ndarray):
    """Host-side specialization of the block-sparse attention pattern.

    Returns a list (one entry per superblock I) of dicts:
      groups: list of groups; each group has
              runs:  [(u, j, i, n)]  one S^T matmul per run (n units wide)
              units: [(u, j, i)]     per-128-col bookkeeping
      masks:  list of ("tri"|"rows"|"zero", group_idx, unit_off, *args)
      pv:     {chunk i: [(group_idx, unit_off, j), ...]}
    """
    ss = _seg_starts(cu)
    n_qb = L // QB
    k_lo_b = [int(ss[i * QB]) // QB for i in range(n_qb)]
    bounds = [int(b) for b in cu[1:-1] if 0 < int(b) < L]

    plan = []
    for I in range(n_qb // SB):
        i0, i1 = SB * I, SB * I + SB - 1
        tiles = []
        for j in range(k_lo_b[i0], i1 + 1):
            qsb = max(i0, j)
            qeb = qsb
            for i in range(qsb, i1 + 1):
                if k_lo_b[i] <= j:
                    qeb = i + 1
                else:
                    break
            if qeb > qsb:
                tiles.append((j, qsb, qeb - qsb))
        # wide tiles first: keeps 2-unit tiles bank-aligned (no splits)
        tiles.sort(key=lambda t: (-t[2], t[0]))

        groups = [{"runs": [], "units": []}]
        masks = []
        pv = {i: [] for i in range(i0, i1 + 1)}
        cursor = 0
        for (j, qsb, n) in tiles:
            if cursor + n > GROUP_UNITS:
                groups.append({"runs": [], "units": []})
                cursor = 0
            g = len(groups) - 1
            u = cursor
            cursor += n
            groups[g]["runs"].append((u, j, qsb, n))
            for c in range(n):
                i = qsb + c
                uu = u + c
                groups[g]["units"].append((uu, j, i))
                pv[i].append((g, uu, j))
                if i == j:
                    masks.append(("tri", g, uu))
                q0u = i * QB
                for b in bounds:
                    if j * QB < b < (j + 1) * QB:
                        c0 = max(0, b - q0u)
                        rb = b - j * QB
                        if c0 < QB:
                            masks.append(("rows", g, uu, c0, rb))
                    elif (j + 1) * QB <= b:
                        c0 = b - q0u
                        if 0 <= c0 < QB:
                            masks.append(("zero", g, uu, c0))
        # PV accumulation order per chunk must be deterministic; sort by j so
        # start/stop flags are simply first/last of the list.
        for i in pv:
            pv[i].sort(key=lambda t: t[2])
        plan.append({"groups": groups, "masks": masks, "pv": pv,
                     "i0": i0, "n_chunks": i1 - i0 + 1})
    return plan


def _build(cu: np.ndarray):
    import concourse.mybir as mybir
    import concourse.tile as tile
    from concourse import bacc
    from concourse.masks import make_identity

    f32 = mybir.dt.float32
    bf16 = mybir.dt.bfloat16
    AF = mybir.ActivationFunctionType
    n_qb = L // QB
    plan = _build_plan(cu)

    nc = bacc.Bacc("TRN2", target_bir_lowering=False, debug=False,
                   num_devices=N_CORES)
    q_d = nc.dram_tensor("q", [L, H_PER_CORE, D], f32, kind="ExternalInput")
    k_d = nc.dram_tensor("k", [L, H_PER_CORE, D], f32, kind="ExternalInput")
    v_d = nc.dram_tensor("v", [L, H_PER_CORE, D], f32, kind="ExternalInput")
    o_d = nc.dram_tensor("out", [L, H_PER_CORE, D], f32, kind="ExternalOutput")

    with tile.TileContext(nc) as tc:
        with (
            tc.tile_pool(name="consts", bufs=1) as consts,
            tc.tile_pool(name="stage", bufs=1) as stage,
            tc.tile_pool(name="big", bufs=1) as big,
            tc.tile_pool(name="psb", bufs=16) as psb,
            tc.tile_pool(name="osb", bufs=2) as osb,
            tc.tile_pool(name="rsb", bufs=4) as rsb,
            tc.tile_pool(name="s_ps", bufs=2, space="PSUM") as s_ps,
            tc.tile_pool(name="o_ps", bufs=2, space="PSUM") as o_ps,
            tc.tile_pool(name="tr_ps", bufs=2, space="PSUM") as tr_ps,
        ):
            ident = consts.tile([128, 128], f32)
            make_identity(nc, ident[:])

            # ---- DMA loads: both heads fused per span (contiguous 1KB per
            # (p, t) element => best HBM efficiency), early spans first ----
            qs = stage.tile([128, n_qb, H_PER_CORE, D], f32, tag="qs")
            ks = stage.tile([128, n_qb, H_PER_CORE, D], f32, tag="ks")
            vs = stage.tile([128, n_qb, H_PER_CORE, D], f32, tag="vs")
            for b0 in range(0, n_qb, 8):
                r = slice(b0 * QB, (b0 + 8) * QB)
                for t_d, t_s in ((qs, q_d), (ks, k_d), (vs, v_d)):
                    nc.sync.dma_start(
                        t_d[:, b0:b0 + 8, :, :],
                        t_s[r, :, :].rearrange("(t p) h d -> p t h d", p=128))

            # per-head prep state; transposes + V casts are emitted on demand
            # inside the main loop so compute tracks DMA arrival
            hstate = []
            for h in range(H_PER_CORE):
                vA = big.tile([128, n_qb, 130], bf16, tag=f"vA{h}")
                nc.gpsimd.memset(vA[:, :, 0:1], 1.0)
                qT = big.tile([128, L], bf16, tag=f"qT{h}")
                kT = big.tile([128, L], bf16, tag=f"kT{h}")
                hstate.append({"vA": vA, "qT": qT, "kT": kT, "done": 0,
                               "pending": None,
                               "ost": {"tile": None, "i0": 0, "filled": 0}})

            def emit_prep(h, need_b):
                hs = hstate[h]
                while hs["done"] < min(need_b, n_qb):
                    b0 = hs["done"]
                    for src, dstT in ((qs, hs["qT"]), (ks, hs["kT"])):
                        trp = tr_ps.tile([128, 4, 128], f32, tag="tr")
                        for t in range(4):
                            nc.tensor.transpose(trp[:, t, :],
                                                src[:, b0 + t, h, :],
                                                ident[:])
                        nc.vector.tensor_copy(
                            dstT[:, b0 * QB:(b0 + 4) * QB], trp[:, :, :])
                    nc.vector.tensor_copy(hs["vA"][:, b0:b0 + 4, 1:129],
                                          vs[:, b0:b0 + 4, h, :])
                    hs["done"] += 4

            def flush_out(h):
                st = hstate[h]["ost"]
                nf = st["filled"]
                if not nf:
                    return
                i0 = st["i0"]
                nc.sync.dma_start(
                    o_d[i0 * QB:(i0 + nf) * QB, h, :].rearrange(
                        "(t p) d -> p t d", p=128),
                    st["tile"][:, 0:nf, :])
                st["tile"] = None
                st["filled"] = 0

            def emit_pv_finalize(h, pend):
                I, ptiles = pend
                sbp = plan[I]
                i0 = sbp["i0"]
                vA = hstate[h]["vA"]
                st = hstate[h]["ost"]
                o_t = o_ps.tile([128, 512], f32, tag="o")
                for c in range(sbp["n_chunks"]):
                    i = i0 + c
                    lst = sbp["pv"][i]
                    for nn, (g, u, j) in enumerate(lst):
                        nc.tensor.matmul(
                            o_t[:, c * 130:c * 130 + 130],
                            ptiles[g][:, u * QB:(u + 1) * QB],
                            vA[:, j, 0:130],
                            start=(nn == 0), stop=(nn == len(lst) - 1))
                rec = rsb.tile([128, 2, 1], f32, tag="r")
                den = o_t[:, 0:260].rearrange("p (c x) -> p c x", c=2)
                nc.vector.reciprocal(rec[:, :, :], den[:, :, 0:1])
                if st["tile"] is None:
                    st["tile"] = osb.tile([128, 2 * SB, 128], f32,
                                          tag=f"ost{h}", name="ost")
                    st["i0"] = i0
                for c in range(sbp["n_chunks"]):
                    nc.vector.tensor_scalar_mul(
                        st["tile"][:, st["filled"] + c, :],
                        o_t[:, c * 130 + 1:c * 130 + 129],
                        rec[:, c, :])
                st["filled"] += sbp["n_chunks"]
                if st["filled"] >= 2 * SB:
                    flush_out(h)

            def emit_groups(h, I):
                sbp = plan[I]
                qT, kT = hstate[h]["qT"], hstate[h]["kT"]
                ptiles = []
                for group in sbp["groups"]:
                    if not group["units"]:
                        continue
                    s_t = s_ps.tile([128, 1024], f32, tag="s")
                    p_t = psb.tile([128, 1024], bf16, tag="p")
                    for (u, j, i, n) in group["runs"]:
                        nc.tensor.matmul(
                            s_t[:, u * QB:(u + n) * QB],
                            kT[:, j * QB:(j + 1) * QB],
                            qT[:, i * QB:(i + n) * QB],
                            start=True, stop=True)
                    gw = (group["units"][-1][0] + 1) * QB
                    nc.scalar.activation(p_t[:, 0:gw], s_t[:, 0:gw],
                                         AF.Exp, scale=SCALE)
                    ptiles.append(p_t)
                # masks (gpsimd), after exp
                for m in sbp["masks"]:
                    kind, g, u = m[0], m[1], m[2]
                    p_t = ptiles[g]
                    sl = p_t[:, u * QB:(u + 1) * QB]
                    if kind == "tri":
                        # keep q >= k: iota = -p + c >= 0
                        nc.gpsimd.affine_select(
                            out=sl, in_=sl,
                            compare_op=mybir.AluOpType.is_ge, fill=0.0,
                            base=0, pattern=[[1, QB]],
                            channel_multiplier=-1)
                    elif kind == "rows":
                        c0, rb = m[3], m[4]
                        sl2 = p_t[:, u * QB + c0:(u + 1) * QB]
                        # keep k-rows >= rb: iota = p - rb >= 0
                        nc.gpsimd.affine_select(
                            out=sl2, in_=sl2,
                            compare_op=mybir.AluOpType.is_ge, fill=0.0,
                            base=-rb, pattern=[[0, QB - c0]],
                            channel_multiplier=1)
                    else:  # "zero"
                        c0 = m[3]
                        nc.gpsimd.memset(p_t[:, u * QB + c0:(u + 1) * QB],
                                         0.0)
                return ptiles

            # ---- main loop: heads interleaved at superblock granularity,
            # software-pipelined by one superblock per head (emit S^T+exp+
            # masks for (h, I), then PV+finalize for (h, I-1)).
            for I, sbp in enumerate(plan):
                for h in range(H_PER_CORE):
                    emit_prep(h, sbp["i0"] + sbp["n_chunks"])
                    ptiles = emit_groups(h, I)
                    if hstate[h]["pending"] is not None:
                        emit_pv_finalize(h, hstate[h]["pending"])
                    hstate[h]["pending"] = (I, ptiles)
            for h in range(H_PER_CORE):
                emit_pv_finalize(h, hstate[h]["pending"])
                flush_out(h)

    nc.compile()
    return nc


def _run(query, key, value, cu_seqlens, trace=False, **spmd_kwargs):
    from concourse import bass_utils

    query = np.ascontiguousarray(np.asarray(query, dtype=np.float32))
    key = np.ascontiguousarray(np.asarray(key, dtype=np.float32))
    value = np.ascontiguousarray(np.asarray(value, dtype=np.float32))
    cu = np.asarray(cu_seqlens, dtype=np.int64)

    nc = _build(cu)
    in_maps = []
    for c in range(N_CORES):
        hs = slice(c * H_PER_CORE, (c + 1) * H_PER_CORE)
        in_maps.append({
            "q": np.ascontiguousarray(query[:, hs, :]),
            "k": np.ascontiguousarray(key[:, hs, :]),
            "v": np.ascontiguousarray(value[:, hs, :]),
        })
    res = bass_utils.run_bass_kernel_spmd(nc, in_maps, list(range(N_CORES)),
                                          trace=trace, **spmd_kwargs)
    out = np.empty((L, H, D), dtype=np.float32)
    for c in range(N_CORES):
        out[:, c * H_PER_CORE:(c + 1) * H_PER_CORE, :] = res.results[c]["out"]
    return out, res


def kernel(query, key, value, cu_seqlens):
    out, _ = _run(query, key, value, cu_seqlens)
    return out
